# revision 8
# baseline (speedup 1.0000x reference)
"""Bass/Trainium2 kernel for nn_Bbox_loss (masked gather + smooth-L1 loss).

Sharding: 8 cores = 4 batches x 2 channel-halves. Core c handles batch
b = c//2 and global channels [3h, 3h+3) with h = c%2 (a contiguous slice
of pred[b]). The host re-lays the per-core pred slice channel-last
(a, d, hh, w, c) so the 3 channel values of one anchor are contiguous;
the device then needs only 3 indirect row-gather DMAs (one per FPN
level, 128 anchors x 3 contiguous f32 each), computes smooth-L1 against
diff, masks padded anchors (via smooth_l1(|e|*mask): smooth_l1(0) == 0),
and reduces to a partial (loss, mask_count). Host sums the 8 partials;
weight = mask_total / 2 (both halves of a batch count the same mask).

The w coordinate is pre-folded on the host into w3b = 3*w + base_l (the
channel-last layout scales flat offsets by 3; base_l is the level's flat
offset). The device clamp max(w3b, 0) keeps padded rows (-1 coords) at a
safe in-range index whose garbage value is masked out.
"""

import numpy as np

import concourse.bacc as bacc
import concourse.bass as bass
import concourse.mybir as mybir
import concourse.tile as tile
from concourse import bass_utils

B, M, A = 4, 128, 3
LEVEL_DIMS = (96, 48, 24)
N_CORES = 8
N_LVL = 3
C_HALF = 3  # channels per half

# per-level flat sizes of the per-core pred slice (9 rows of S^3 each)
_LVL_SIZES = tuple(9 * s**3 for s in LEVEL_DIMS)
_LVL_BASE = (0, _LVL_SIZES[0], _LVL_SIZES[0] + _LVL_SIZES[1])
NP_TOT = sum(_LVL_SIZES)
# 2-D view of the flat pred slice (DMA APs need >=2 dims; flat order kept,
# gather indices stay flat element indices because coef(axis=1) == 1)
PRED_COLS = 512
PRED_ROWS = NP_TOT // PRED_COLS
assert PRED_ROWS * PRED_COLS == NP_TOT

# meta input columns (all int32; diff is f32 bit-cast)
# coords are level-major (l*4 + comp); comp = (a, d, h, w3b)
_C_COORD = 0    # 12 cols
_C_SVEC = 12    # 3 cols: per-level S (Horner multiplier, steps 1-2)
_C_SVEC3 = 15   # 3 cols: per-level 3*S (last Horner multiplier)
_C_DIFF = 18    # 9 cols: diff values (f32 bits), col = 3*l + c
META_COLS = 27

_F32 = mybir.dt.float32
_I32 = mybir.dt.int32

_BUILD_CACHE = {}


def _build():
    """Build + compile the (shared SPMD) Bass module once per process."""
    if "nc" in _BUILD_CACHE:
        return _BUILD_CACHE["nc"]

    nc = bacc.Bacc(
        "TRN2", target_bir_lowering=False, debug=False, num_devices=N_CORES
    )
    pred_h = nc.dram_tensor(
        "pred", [PRED_ROWS, PRED_COLS], _F32, kind="ExternalInput"
    )
    meta_h = nc.dram_tensor("meta", [M, META_COLS], _I32, kind="ExternalInput")
    out_h = nc.dram_tensor("out", [1, 2], _F32, kind="ExternalOutput")

    op = mybir.AluOpType
    with tile.TileContext(nc) as tc:
        with (
            tc.tile_pool(name="sb", bufs=1) as pool,
            tc.tile_pool(name="pp", bufs=1, space="PSUM") as psum_pool,
        ):
            ct = pool.tile([M, META_COLS], _I32)
            nc.sync.dma_start(out=ct[:], in_=meta_h.ap())

            sv = ct[:, _C_SVEC : _C_SVEC + 3]
            sv3 = ct[:, _C_SVEC3 : _C_SVEC3 + 3]
            dt = ct[:, _C_DIFF : _C_DIFF + 9].bitcast(_F32)
            coords = ct[:, _C_COORD : _C_COORD + 12].rearrange(
                "p (l c) -> p l c", c=4
            )

            ps = pool.tile([M, 2], _F32)

            # clamped coords
            cm_t = pool.tile([M, 12], _I32)
            nc.vector.tensor_scalar(
                out=cm_t[:],
                in0=ct[:, _C_COORD : _C_COORD + 12],
                scalar1=0,
                scalar2=None,
                op0=op.max,
            )
            cm = cm_t[:].rearrange("p (l c) -> p l c", c=4)

            # ridx = ((a*S + d)*S + h)*(3S) + (3w + base)  (per level column)
            lin = pool.tile([M, N_LVL], _I32)
            nc.vector.tensor_tensor(
                out=lin[:], in0=cm[:, :, 0], in1=sv, op=op.mult
            )
            nc.vector.tensor_tensor(
                out=lin[:], in0=lin[:], in1=cm[:, :, 1], op=op.add
            )
            nc.vector.tensor_tensor(
                out=lin[:], in0=lin[:], in1=sv, op=op.mult
            )
            nc.vector.tensor_tensor(
                out=lin[:], in0=lin[:], in1=cm[:, :, 2], op=op.add
            )
            nc.vector.tensor_tensor(
                out=lin[:], in0=lin[:], in1=sv3, op=op.mult
            )
            nc.vector.tensor_tensor(
                out=lin[:], in0=lin[:], in1=cm[:, :, 3], op=op.add
            )

            # mask[p, l] = coord_a > -1 (pre-clamp), as f32 0/1
            mask = pool.tile([M, N_LVL], _F32)
            nc.vector.tensor_scalar(
                out=mask[:],
                in0=coords[:, :, 0],
                scalar1=-1,
                scalar2=None,
                op0=op.is_gt,
            )
            nc.vector.tensor_reduce(
                out=ps[:, 1:2],
                in_=mask[:],
                axis=mybir.AxisListType.X,
                op=op.add,
            )

            # 3 row-gathers: one per level, 128 rows x 3 contiguous f32
            gt = pool.tile([M, 9], _F32)
            for l in range(N_LVL):
                nc.gpsimd.indirect_dma_start(
                    out=gt[:, 3 * l : 3 * l + 3],
                    out_offset=None,
                    in_=pred_h.ap(),
                    in_offset=bass.IndirectOffsetOnAxis(
                        ap=lin[:, l : l + 1], axis=1
                    ),
                )

            # masked smooth-L1 (smooth_l1(|e|*mask): padded rows -> 0)
            e = pool.tile([M, 9], _F32)
            nc.vector.tensor_sub(out=e[:], in0=gt[:], in1=dt)
            ae = pool.tile([M, 9], _F32)
            nc.vector.scalar_tensor_tensor(
                out=ae[:], in0=e[:], scalar=-1.0, in1=e[:],
                op0=op.mult, op1=op.max,
            )
            maskb = mask[:, :, None].to_broadcast([M, N_LVL, C_HALF])
            aev = ae[:].rearrange("p (l c) -> p l c", c=3)
            nc.vector.tensor_tensor(out=aev, in0=aev, in1=maskb, op=op.mult)
            mt = pool.tile([M, 9], _F32)
            nc.vector.tensor_scalar(
                out=mt[:], in0=ae[:], scalar1=1.0, scalar2=None, op0=op.min
            )
            hq = pool.tile([M, 9], _F32)
            nc.vector.scalar_tensor_tensor(
                out=hq[:], in0=mt[:], scalar=0.5, in1=mt[:],
                op0=op.mult, op1=op.mult,
            )
            t1 = pool.tile([M, 9], _F32)
            nc.vector.scalar_tensor_tensor(
                out=t1[:], in0=mt[:], scalar=-1.0, in1=ae[:],
                op0=op.mult, op1=op.add,
            )
            v = pool.tile([M, 9], _F32)
            nc.vector.scalar_tensor_tensor(
                out=v[:], in0=t1[:], scalar=1.0, in1=hq[:],
                op0=op.mult, op1=op.add,
                accum_out=ps[:, 0:1],
            )

            # partition reduce via matmul with ones
            ones = pool.tile([M, 1], _F32)
            nc.vector.memset(ones[:], 1.0)
            acc = psum_pool.tile([1, 2], _F32)
            nc.tensor.matmul(
                out=acc[:], lhsT=ones[:], rhs=ps[:], start=True, stop=True
            )
            osb = pool.tile([1, 2], _F32)
            nc.vector.tensor_copy(out=osb[:], in_=acc[:])
            nc.sync.dma_start(out=out_h.ap(), in_=osb[:])

    nc.compile()
    _BUILD_CACHE["nc"] = nc
    return nc


def _shard(inputs):
    """Build the 8 per-core input maps from the full inputs."""
    preds = [np.ascontiguousarray(inputs[f"pred_l{l}"], dtype=np.float32)
             for l in range(N_LVL)]
    coords = [np.ascontiguousarray(inputs[f"coord_l{l}"], dtype=np.int32)
              for l in range(N_LVL)]
    diffs = [np.ascontiguousarray(inputs[f"diff_l{l}"], dtype=np.float32)
             for l in range(N_LVL)]

    in_maps = []
    for c in range(N_CORES):
        b, h = divmod(c, 2)
        # channel-last relayout: block (3c, 3a, S^3) -> (3a, S^3, 3c)
        blocks = []
        for l in range(N_LVL):
            s3 = LEVEL_DIMS[l] ** 3
            blk = preds[l][b, 9 * h : 9 * h + 9].reshape(C_HALF, A, s3)
            blocks.append(blk.transpose(1, 2, 0).reshape(-1))
        pred_flat = np.concatenate(blocks).reshape(PRED_ROWS, PRED_COLS)

        meta = np.empty((M, META_COLS), dtype=np.int32)
        for l in range(N_LVL):
            meta[:, _C_COORD + 4 * l : _C_COORD + 4 * l + 4] = coords[l][b]
            # fold *3 + level base into the w coordinate (stays <0 for
            # padded rows only when 3*(-1)+base < 0, i.e. level 0 -> the
            # device clamp keeps every padded index in range)
            meta[:, _C_COORD + 4 * l + 3] = (
                coords[l][b][:, 3] * 3 + _LVL_BASE[l]
            )
            meta[:, _C_SVEC + l] = LEVEL_DIMS[l]
            meta[:, _C_SVEC3 + l] = 3 * LEVEL_DIMS[l]
            meta[:, _C_DIFF + 3 * l : _C_DIFF + 3 * l + 3] = (
                diffs[l][b, :, 3 * h : 3 * h + 3].view(np.int32)
            )
        in_maps.append({"pred": pred_flat, "meta": meta})
    return in_maps


def run(inputs, trace=False, **kw):
    nc = _build()
    in_maps = _shard(inputs)
    res = bass_utils.run_bass_kernel_spmd(
        nc, in_maps, core_ids=list(range(N_CORES)), trace=trace, **kw
    )
    partials = np.stack([res.results[c]["out"][0] for c in range(N_CORES)])
    loss = np.float32(partials[:, 0].sum())
    weight = np.float32(partials[:, 1].sum() / 2.0)
    return (
        np.array([loss], dtype=np.float32),
        np.array([weight], dtype=np.float32),
    ), res


def kernel(**inputs):
    out, _ = run(inputs, trace=False)
    return out
